# revision 46
# baseline (speedup 1.0000x reference)
"""GCN (3x GCNConv + global mean pool + linear) on 8 Trainium2 NeuronCores.

Strategy (dst-sharded message passing):
  - Nodes are sharded n/8 per core; each core's nodes are permuted
    (degree-balanced) into supertiles of 128 (one PSUM bank each).
  - Edges are partitioned by dst core and packed into (supertile, class)
    tiles of 128 edges; class = which signed-int16-indexable half of the
    node table the src row lives in (dma_gather indices are int16).
  - Per layer: dma_gather pulls 256B fp16 rows of the scaled node table
    h_hat = dinv*h from HBM, round-robined over 4 SWDGE queues (the
    per-queue descriptor-generation ucode is the kernel bottleneck);
    TensorE computes the transposed scatter-add aggT = gt^T @ st per edge
    tile (gt = gathered rows as stationary weights, st = [128e, 128slot]
    one-hot*ew, built on DVE in [128, slot, tile] layout so all operands
    are packed fp16 and hit the DVE 2x/4x mode). Each supertile's PSUM
    bank holds exactly ONE accumulation group - concurrently open groups
    on the same bank partitions corrupt PSUM on HW.
  - Norm + self loop folded algebraically:
        gcn_conv(h) = (dinv * (A_ew @ h_hat + h_hat)) @ W + b
    so per supertile (all in transposed [feat, node] layout): uT =
    (aggT + h_hatT)*dinv_rep, f32 GEMM hT = W^T @ uT, bias(+relu) on ACT,
    table write via one PE transpose + dinv scale on the Scalar engine.
    h_hatT persists in SBUF fp16 as next layer's self-loop term.
  - The inter-layer table AllGather is split into 4 sup-range collectives
    fired as their range's shard writes complete (the global table row
    layout is split-major so each sub-collective lands contiguously),
    overlapping the collective with the remaining scatter work.
  - Pooling: matmul with host-built P (1[batch==g]) accumulated over
    supertiles -> AllReduce -> final linear on-device -> out [64, 5] f32.
"""

import os
import sys
import numpy as np

for _p in ("/opt/trn_rl_repo", "/root/.axon_site/_ro/trn_rl_repo"):
    if os.path.isdir(_p) and _p not in sys.path:
        sys.path.insert(0, _p)

N_CORES = 8
N_GRAPHS = 64
HID = 128
N_CLASS = 5
F_IN = 7
F_PAD = 8
WIN = 32
SUP = 128
NSPLIT = 4
GROUP_SUPS = 4
CHUNK_MAX = 32
GATH_BUFS = 6
N_QUEUES = 4  # SWDGE queues; gathers round-robin across them (~3.5x issue rate)
SINGLE_PACKET = False  # True requires chunks of <= 8 tiles (1024 descriptors)
IDX_CAP = 32768


def _group_ranks(keys, n_keys):
    """rank of each element within its key group (keys int array)."""
    nk = len(keys)
    if nk == 0:
        return np.zeros(0, dtype=np.int64)
    order = np.argsort(keys, kind="stable")
    sk = keys[order]
    is_new = np.r_[True, sk[1:] != sk[:-1]]
    gs_idx = np.nonzero(is_new)[0]
    gs = np.repeat(gs_idx, np.diff(np.r_[gs_idx, nk]))
    rank = np.empty(nk, dtype=np.int64)
    rank[order] = np.arange(nk) - gs
    return rank


class Plan:
    pass


def build_plan(x, edge_index, edge_attr, batch, n_cores=N_CORES, n_graphs=N_GRAPHS):
    """Host-side sharding/layout planning: pure permutation / zero-padding of
    inputs, no arithmetic on float data."""
    p = Plan()
    n = x.shape[0]
    assert n % n_cores == 0
    npc = n // n_cores
    nsup = (npc + SUP - 1) // SUP
    padc = nsup * SUP
    npad = n_cores * padc
    nwin = padc // WIN
    p.n, p.npc, p.nsup, p.padc, p.npad, p.nwin = n, npc, nsup, padc, npad, nwin
    p.n_cores, p.n_graphs = n_cores, n_graphs
    base_hi = max(0, npad - IDX_CAP)
    p.base_hi = base_hi

    src = np.asarray(edge_index[0], dtype=np.int64)
    dst = np.asarray(edge_index[1], dtype=np.int64)
    ew = np.asarray(edge_attr, dtype=np.float32)
    batch = np.asarray(batch, dtype=np.int64)

    # ---- window assignment (degree-balanced snake over sorted degrees) ----
    indeg = np.bincount(dst, minlength=n)
    prow = np.empty(n, dtype=np.int64)
    win_all = np.empty(n, dtype=np.int64)
    slot_all = np.empty(n, dtype=np.int64)
    for c in range(n_cores):
        lo = c * npc
        order = np.argsort(-indeg[lo : lo + npc], kind="stable")
        pos = np.empty(npc, dtype=np.int64)
        pos[order] = np.arange(npc)
        rnd = pos // nwin
        off = pos % nwin
        w = np.where(rnd % 2 == 0, off, nwin - 1 - off)
        plid = (w // 4) * SUP + (w % 4) * WIN + rnd
        prow[lo : lo + npc] = c * padc + plid
        win_all[lo : lo + npc] = w
        slot_all[lo : lo + npc] = rnd

    # split-major global table layout: the AllGather is split into NSPLIT
    # sup-ranges fired as they complete; each sub-range output must land
    # contiguously, so row = base_k + core*len_k + (plid - off_k).
    ends = sorted(
        set((nsup * k + NSPLIT - 1) // NSPLIT for k in range(1, NSPLIT + 1))
    )
    starts = [0] + ends[:-1]
    p.ag_splits = list(zip(starts, ends))
    split_of_sup = np.zeros(nsup, dtype=np.int64)
    base_arr = np.zeros(len(p.ag_splits), dtype=np.int64)
    r0_arr = np.zeros(len(p.ag_splits), dtype=np.int64)
    len_arr = np.zeros(len(p.ag_splits), dtype=np.int64)
    acc = 0
    for k, (r0, r1) in enumerate(p.ag_splits):
        split_of_sup[r0:r1] = k
        base_arr[k] = acc
        r0_arr[k] = r0
        len_arr[k] = r1 - r0
        acc += n_cores * (r1 - r0) * SUP
    p.ag_base = [int(b) for b in base_arr]
    plid_all = prow % padc
    core_all = prow // padc
    k_all = split_of_sup[plid_all // SUP]
    prow = (
        base_arr[k_all]
        + core_all * len_arr[k_all] * SUP
        + (plid_all - r0_arr[k_all] * SUP)
    )
    p.prow = prow
    p.plid_all = plid_all

    # ---- per-core-supertile tile budgets (uniform across cores) ----
    # Edges are packed per dst supertile (128 slots) so each PSUM bank holds
    # exactly one accumulation group (concurrent groups on the same bank
    # partitions are a HW hazard).
    ecore = dst // npc
    esrc_prow = prow[src]
    can_lo = esrc_prow < IDX_CAP
    can_hi = esrc_prow >= base_hi
    esup = plid_all[dst] // SUP

    flo_cs = np.zeros((n_cores, nsup), dtype=np.int64)
    fhi_cs = np.zeros((n_cores, nsup), dtype=np.int64)
    tot_cs = np.zeros((n_cores, nsup), dtype=np.int64)
    np.add.at(tot_cs, (ecore, esup), 1)
    np.add.at(flo_cs, (ecore[~can_hi], esup[~can_hi]), 1)
    np.add.at(fhi_cs, (ecore[~can_lo], esup[~can_lo]), 1)

    t_lo = np.max((flo_cs + 127) // 128, axis=0)
    t_hi = np.max((fhi_cs + 127) // 128, axis=0)
    grow = np.maximum(np.max(tot_cs, axis=0) - (t_lo + t_hi) * 128, 0)
    t_lo = t_lo + (grow + 127) // 128
    t_lo = np.maximum(t_lo, (t_lo + t_hi) == 0)
    p.t_lo, p.t_hi = t_lo, t_hi

    # ---- global tile order: groups of supertiles, class runs within group --
    n_groups = (nsup + GROUP_SUPS - 1) // GROUP_SUPS
    p.n_groups = n_groups
    tiles = []  # (sup, cls)
    chunks = []  # (tile_start, ntiles, cls)
    groups = []  # (sup_start, nsups, [chunk idx], (t0, t1))
    for g in range(n_groups):
        s0 = g * GROUP_SUPS
        ns = min(GROUP_SUPS, nsup - s0)
        g_t0 = len(tiles)
        g_chunks = []
        for cls in (0, 1):
            run_t0 = len(tiles)
            for s in range(s0, s0 + ns):
                tc = int(t_lo[s]) if cls == 0 else int(t_hi[s])
                tiles.extend((s, cls) for _ in range(tc))
            nrun = len(tiles) - run_t0
            t0 = run_t0
            while nrun > 0:
                take = min(CHUNK_MAX, nrun)
                g_chunks.append(len(chunks))
                chunks.append((t0, take, cls))
                t0 += take
                nrun -= take
        groups.append((s0, ns, g_chunks, (g_t0, len(tiles))))
    p.tiles, p.chunks, p.groups = tiles, chunks, groups
    p.tot = len(tiles)
    p.chunk_of = np.zeros(p.tot, dtype=np.int64)
    for ci, (t0, ntl, _c) in enumerate(chunks):
        p.chunk_of[t0 : t0 + ntl] = ci

    # first/last tile of each supertile (for PSUM start/stop flags)
    tw = np.array([t[0] for t in tiles])
    p.first_of_sup = np.zeros(p.tot, dtype=bool)
    p.last_of_sup = np.zeros(p.tot, dtype=bool)
    for s in range(nsup):
        ids = np.nonzero(tw == s)[0]
        p.first_of_sup[ids.min()] = True
        p.last_of_sup[ids.max()] = True
    # first tile index of each (sup, cls) run
    t_off = {}
    for t, (s, cls) in enumerate(tiles):
        t_off.setdefault((s, cls), t)

    counts = np.bincount(batch, minlength=n_graphs).astype(np.float32)
    p.counts = counts
    L = int(indeg.max()) + 1
    p.L = L

    # ---- per-core arrays ----
    p.per_core = []
    for c in range(n_cores):
        m = ecore == c
        ed, ee = dst[m], ew[m]
        eprow = esrc_prow[m]
        esup_c = esup[m]
        eslot = plid_all[ed] % SUP
        e_can_hi = can_hi[m]
        e_can_lo = can_lo[m]
        ne = len(ed)

        # per-edge class: fill lo up to its target, rest hi
        ecls = np.full(ne, -1, dtype=np.int64)
        ecls[~e_can_hi] = 0
        ecls[~e_can_lo] = 1
        free = ecls == -1
        # per-supertile lo target
        tot_s = np.bincount(esup_c, minlength=nsup)
        flo_s = np.bincount(esup_c[~e_can_hi], minlength=nsup)
        lo_target = np.maximum(flo_s, tot_s - t_hi * 128)
        lo_target = np.minimum(lo_target, t_lo * 128)
        # rank of free edges within supertile
        fidx = np.nonzero(free)[0]
        frank = _group_ranks(esup_c[fidx], nsup)
        to_lo = frank < (lo_target - flo_s)[esup_c[fidx]]
        ecls[fidx[to_lo]] = 0
        ecls[fidx[~to_lo]] = 1

        # slot position within (sup, cls) run
        key = esup_c * 2 + ecls
        k = _group_ranks(key, nsup * 2)
        t_off_arr = np.zeros((nsup, 2), dtype=np.int64)
        for (sv, cv), tv in t_off.items():
            t_off_arr[sv, cv] = tv
        run0 = t_off_arr[esup_c, ecls]
        t_of_e = run0 + k // 128
        p_of_e = k % 128

        idx_arr = np.zeros((p.tot, 128), dtype=np.int16)
        slot_arr = np.zeros((p.tot, 128), dtype=np.float16)
        ew_arr = np.zeros((p.tot, 128), dtype=np.float16)
        rel = eprow - np.where(ecls == 1, base_hi, 0)
        assert rel.min() >= 0 and rel.max() < IDX_CAP
        idx_arr[t_of_e, p_of_e] = rel.astype(np.int16)
        slot_arr[t_of_e, p_of_e] = eslot.astype(np.float16)
        ew_arr[t_of_e, p_of_e] = ee.astype(np.float16)

        # wrapped idx layout [16, tot*8], replicated to [128, tot*8]
        idx16 = np.zeros((16, p.tot * 8), dtype=np.int16)
        for ppart in range(128):
            idx16[ppart % 16, np.arange(p.tot) * 8 + ppart // 16] = idx_arr[:, ppart]
        idx128 = np.ascontiguousarray(np.tile(idx16, (8, 1)))

        dstslot = np.ascontiguousarray(slot_arr.T)  # [128, tot] fp16
        ews = np.ascontiguousarray(ew_arr.T)  # [128, tot] fp16

        # deg accumulation layout [128, nsup*L] (plid -> [plid%128, plid//128 * L + k])
        ewp = np.zeros((128, nsup * L), dtype=np.float32)
        plid_own = plid_all[ed].astype(np.int64)
        kk = _group_ranks(plid_own, padc)
        ewp[plid_own % 128, (plid_own // 128) * L + kk] = ee
        p.per_core.append(dict(idx128=idx128, dstslot=dstslot, ews=ews, ewp=ewp))

    # ---- node-indexed arrays ----
    # xpad [npad, F_PAD] f32 (global, by prow) — same for every core
    xf = np.asarray(x, dtype=np.float32)
    xpad = np.zeros((npad, F_PAD), dtype=np.float32)
    xpad[prow, :F_IN] = xf
    p.xpad = xpad
    # per-core x_own [padc, F_PAD]
    p.x_own = []
    p.pmat = []
    for c in range(n_cores):
        lo = c * npc
        xo = np.zeros((padc, F_PAD), dtype=np.float32)
        plid = plid_all[lo : lo + npc]
        xo[plid, :F_IN] = xf[lo : lo + npc]
        p.x_own.append(xo)
        # pool matrix [128, nsup*64] fp16: 1.0 at [plid%128, (plid//128)*G + batch]
        pm = np.zeros((128, nsup * n_graphs), dtype=np.float16)
        pm[plid % 128, (plid // 128) * n_graphs + batch[lo : lo + npc]] = 1.0
        p.pmat.append(np.ascontiguousarray(pm))

    # iota const [128, SUP, CHUNK_MAX] fp16: value d at (p, d, t)
    p.iota = np.ascontiguousarray(
        np.broadcast_to(
            np.repeat(np.arange(SUP, dtype=np.float16), CHUNK_MAX),
            (128, SUP * CHUNK_MAX),
        )
    )
    p.identity = np.eye(128, dtype=np.float32)
    return p


def build_weight_arrays(p, W1, b1, W2, b2, W3, b3, Wl, bl):
    """Zero-pad / reshape weights (no arithmetic)."""
    w1p = np.zeros((F_PAD, HID), dtype=np.float32)
    w1p[:F_IN] = np.asarray(W1, dtype=np.float32)
    a = dict(
        w1=w1p,
        w2=np.asarray(W2, dtype=np.float32),
        w3=np.asarray(W3, dtype=np.float32),
        wl=np.asarray(Wl, dtype=np.float32),
        b1=np.asarray(b1, dtype=np.float32).reshape(HID, 1),
        b2=np.asarray(b2, dtype=np.float32).reshape(HID, 1),
        b3=np.asarray(b3, dtype=np.float32).reshape(HID, 1),
        blrep=np.ascontiguousarray(
            np.broadcast_to(np.asarray(bl, dtype=np.float32), (p.n_graphs, N_CLASS))
        ),
        invc=(1.0 / np.maximum(p.counts, 1.0)).reshape(p.n_graphs, 1),
    )
    return a


# ----------------------------------------------------------------------------
# Device program
# ----------------------------------------------------------------------------
def build_program(p, enable_asserts=False):
    import dataclasses
    import concourse.bass as bass
    import concourse.bacc as bacc
    import concourse.tile as tile
    import concourse.mybir as mybir

    dt = mybir.dt
    f32, f16, i16 = dt.float32, dt.float16, dt.int16
    Alu = mybir.AluOpType
    Act = mybir.ActivationFunctionType
    G = p.n_graphs
    rg = [list(range(p.n_cores))]

    def bc(ap, nrep):
        """append a step-0 (broadcast) innermost free dim to an AP"""
        return dataclasses.replace(ap, ap=list(ap.ap) + [[0, nrep]])

    def bcmid(ap, nrep):
        """insert a step-0 (broadcast) free dim after the partition dim"""
        return dataclasses.replace(
            ap, ap=[ap.ap[0], [0, nrep]] + list(ap.ap[1:])
        )

    nc = bacc.Bacc(
        "TRN2",
        target_bir_lowering=False,
        debug=False,
        enable_asserts=enable_asserts,
        num_devices=p.n_cores,
        num_swdge_queues=N_QUEUES,
    )

    # ---- DRAM tensors ----
    xown_d = nc.dram_tensor("x_own", [p.padc, F_PAD], f32, kind="ExternalInput")
    ewp_d = nc.dram_tensor("ewp", [128, p.nsup, p.L], f32, kind="ExternalInput")
    idx_d = nc.dram_tensor("idx", [128, p.tot * 8], i16, kind="ExternalInput")
    dstslot_d = nc.dram_tensor("dstslot", [128, p.tot], f16, kind="ExternalInput")
    ews_d = nc.dram_tensor("ews", [128, p.tot], f16, kind="ExternalInput")
    iota_d = nc.dram_tensor("iota", [128, SUP, CHUNK_MAX], f16, kind="ExternalInput")
    pmat_d = nc.dram_tensor("pmat", [128, p.nsup * G], f16, kind="ExternalInput")
    w1_d = nc.dram_tensor("w1", [F_PAD, HID], f32, kind="ExternalInput")
    w2_d = nc.dram_tensor("w2", [HID, HID], f32, kind="ExternalInput")
    w3_d = nc.dram_tensor("w3", [HID, HID], f32, kind="ExternalInput")
    wl_d = nc.dram_tensor("wl", [HID, N_CLASS], f32, kind="ExternalInput")
    b1_d = nc.dram_tensor("b1", [HID, 1], f32, kind="ExternalInput")
    b2_d = nc.dram_tensor("b2", [HID, 1], f32, kind="ExternalInput")
    b3_d = nc.dram_tensor("b3", [HID, 1], f32, kind="ExternalInput")
    invc_d = nc.dram_tensor("invc", [G, 1], f32, kind="ExternalInput")
    blrep_d = nc.dram_tensor("blrep", [G, N_CLASS], f32, kind="ExternalInput")
    ident_d = nc.dram_tensor("ident", [128, 128], f32, kind="ExternalInput")
    out_d = nc.dram_tensor("out", [G, N_CLASS], f32, kind="ExternalOutput")

    table1_d = nc.dram_tensor(
        "table1", [p.npad, HID], f16, kind="Internal", addr_space="Shared"
    )
    agin_d = nc.dram_tensor("agin", [p.padc, HID], f16, kind="Internal")
    dvin_d = nc.dram_tensor("dvin", [p.padc], f16, kind="Internal")
    table2_d = nc.dram_tensor(
        "table2", [p.npad, HID], f16, kind="Internal", addr_space="Shared"
    )
    table3_d = nc.dram_tensor(
        "table3", [p.npad, HID], f16, kind="Internal", addr_space="Shared"
    )
    arin_d = nc.dram_tensor("arin", [128, G], f32, kind="Internal")
    arout_d = nc.dram_tensor(
        "arout", [128, G], f32, kind="Internal", addr_space="Shared"
    )

    ncols = p.npad // 128  # node-table columns in [128, ncols] layout

    with tile.TileContext(nc) as tc:
        with (
            tc.tile_pool(name="const", bufs=1) as cpool,
            tc.tile_pool(name="gath", bufs=GATH_BUFS) as gpool,
            tc.tile_pool(name="sbld", bufs=4) as spool,
            tc.tile_pool(name="stage", bufs=3) as stpool,
            tc.tile_pool(name="psagg", bufs=GROUP_SUPS + 1, space="PSUM") as psagg,
            tc.tile_pool(name="psstg", bufs=2, space="PSUM") as psstg,
            tc.tile_pool(name="psacc", bufs=1, space="PSUM") as psacc,
        ):
            # ---- persistent SBUF tiles ----
            ident = cpool.tile([128, 128], f32, tag="ident")
            nc.sync.dma_start(ident[:, :], ident_d[:, :])
            ident16 = cpool.tile([128, 128], f16, tag="ident16")
            nc.vector.tensor_copy(ident16[:, :], ident[:, :])
            w1 = cpool.tile([F_PAD, HID], f32, tag="w1")
            nc.sync.dma_start(w1[:, :], w1_d[:, :])
            w2 = cpool.tile([HID, HID], f32, tag="w2")
            nc.sync.dma_start(w2[:, :], w2_d[:, :])
            w3 = cpool.tile([HID, HID], f32, tag="w3")
            nc.sync.dma_start(w3[:, :], w3_d[:, :])
            wl = cpool.tile([HID, N_CLASS], f32, tag="wl")
            nc.sync.dma_start(wl[:, :], wl_d[:, :])
            b1 = cpool.tile([HID, 1], f32, tag="b1")
            nc.sync.dma_start(b1[:, :], b1_d[:, :])
            b2 = cpool.tile([HID, 1], f32, tag="b2")
            nc.sync.dma_start(b2[:, :], b2_d[:, :])
            b3 = cpool.tile([HID, 1], f32, tag="b3")
            nc.sync.dma_start(b3[:, :], b3_d[:, :])
            invc = cpool.tile([G, 1], f32, tag="invc")
            nc.sync.dma_start(invc[:, :], invc_d[:, :])
            blrep = cpool.tile([G, N_CLASS], f32, tag="blrep")
            nc.sync.dma_start(blrep[:, :], blrep_d[:, :])
            iota = cpool.tile([128, SUP, CHUNK_MAX], f16, tag="iota")
            nc.sync.dma_start(iota[:, :, :], iota_d[:, :, :])
            pmat = cpool.tile([128, p.nsup * G], f16, tag="pmat")
            nc.sync.dma_start(pmat[:, :], pmat_d[:, :])
            idx_sb = cpool.tile([128, p.tot * 8], i16, tag="idx")
            nc.sync.dma_start(idx_sb[:, :], idx_d[:, :])
            dstslot = cpool.tile([128, p.tot], f16, tag="dstslot")
            nc.sync.dma_start(dstslot[:, :], dstslot_d[:, :])
            ews = cpool.tile([128, p.tot], f16, tag="ews")
            nc.sync.dma_start(ews[:, :], ews_d[:, :])
            dinv_own = cpool.tile([128, p.nsup], f32, tag="dinv_own")
            # transposed-layout persistents: [feat, node], node = s*128+j.
            # hfa/hfb hold h*dinv (the fp16 gather-table values); the layer
            # update uses u = (agg + h*dinv)*dinv.
            dinv_rep = cpool.tile([128, p.padc], f16, tag="dinv_rep")
            xhatT = cpool.tile([F_PAD, p.padc], f16, tag="xhatT")
            hfa = cpool.tile([128, p.nsup * HID], f16, tag="hfa")
            hfb = cpool.tile([128, p.nsup * HID], f16, tag="hfb")


            # ---- phase 0: deg -> dinv; x_hat shard -> AllGather table1 ----
            with (
                tc.tile_pool(name="ph0", bufs=2) as ph0,
                tc.tile_pool(name="ph0c", bufs=1) as ph0c,
            ):
                ewp_t = ph0c.tile([128, p.nsup, p.L], f32, tag="ewp")
                nc.sync.dma_start(ewp_t[:, :, :], ewp_d[:, :, :])
                deg = ph0.tile([128, p.nsup], f32, tag="deg")
                nc.vector.tensor_reduce(
                    deg[:, :], ewp_t[:, :, :], mybir.AxisListType.X, Alu.add
                )
                nc.vector.tensor_scalar(deg[:, :], deg[:, :], 1.0, None, Alu.add)
                nc.scalar.sqrt(deg[:, :], deg[:, :])
                nc.vector.reciprocal(dinv_own[:, :], deg[:, :])
                # x_hat = x_own * dinv; fp16 shard (zero-padded to HID cols)
                xo = ph0.tile([128, p.nsup, F_PAD], f32, tag="xo")
                nc.sync.dma_start(
                    xo[:, :, :], xown_d[:, :].rearrange("(s q) f -> q s f", q=128)
                )
                dvb = bc(dinv_own[:, :], F_PAD)
                xdv = ph0.tile([128, p.nsup, F_PAD], f32, tag="xdv")
                nc.vector.tensor_tensor(xdv[:, :, :], xo[:, :, :], dvb, Alu.mult)
                maxsp = max(r1 - r0 for (r0, r1) in p.ag_splits)
                for k, (r0, r1) in enumerate(p.ag_splits):
                    ns_k = r1 - r0
                    xh2 = ph0.tile([128, maxsp, HID], f16, tag="xh2")
                    nc.vector.memset(xh2[:, :ns_k, :], 0.0)
                    nc.vector.tensor_copy(
                        xh2[:, :ns_k, 0:F_PAD], xdv[:, r0:r1, :]
                    )
                    nc.sync.dma_start(
                        agin_d[:, :].rearrange("(s q) f -> q s f", q=128)[
                            :, r0:r1, :
                        ],
                        xh2[:, :ns_k, :],
                    )
                    base = p.ag_base[k]
                    blen = p.n_cores * ns_k * SUP
                    nc.gpsimd.collective_compute(
                        "AllGather",
                        Alu.bypass,
                        replica_groups=rg,
                        ins=[agin_d[r0 * SUP : r1 * SUP, :]],
                        outs=[table1_d[base : base + blen, :]],
                    )
                # dinv_rep[p, s*128+j] = dinv(node s*128+j) for all p, via
                # dvin roundtrip + gpsimd partition broadcast
                dinv16 = ph0.tile([128, p.nsup], f16, tag="dinv16")
                nc.vector.tensor_copy(dinv16[:, :], dinv_own[:, :])
                nc.sync.dma_start(
                    dvin_d[:].rearrange("(s q) -> q s", q=128), dinv16[:, :]
                )
                dinv_row = ph0c.tile([1, p.padc], f16, tag="dinv_row")
                nc.sync.dma_start(
                    dinv_row[:, :], dvin_d[:].rearrange("(a x) -> a x", a=1)
                )
                nc.gpsimd.partition_broadcast(
                    dinv_rep[:, :], dinv_row[0:1, :], channels=128
                )
                # xhatT = (x * dinv)^T fp16 via f32 PE transposes of xdv
                for s in range(p.nsup):
                    stg0 = psstg.tile([128, 320], f32, tag="stg")
                    nc.tensor.transpose(
                        stg0[0:F_PAD, 0:128], xdv[:, s, :], ident[:, :]
                    )
                    nc.vector.tensor_copy(
                        xhatT[:, s * 128 : (s + 1) * 128], stg0[0:F_PAD, 0:128]
                    )

            # ---- persistent PSUM tiles ----
            pacc = psacc.tile([128, G], f32, tag="pacc")

            # ---- layers ----
            # table AllGather split into sub-ranges fired as their last
            # contributing supertile's agin write is issued, overlapping the
            # collective with the remaining scatter/epilogue work
            ag_ends = [r1 for (_r0, r1) in p.ag_splits]
            layers = [
                (0, table1_d, w1, b1, True, table2_d, xhatT, hfa),
                (1, table2_d, w2, b2, True, table3_d, hfa, hfb),
                (2, table3_d, w3, b3, False, None, hfb, None),
            ]
            for li, tab_d, w_sb, b_sb, relu, tab_next, hin, hfout in layers:
                fdim = F_PAD if li == 0 else HID
                lo_view = tab_d[0 : min(p.npad, IDX_CAP), :]
                hi_view = tab_d[p.base_hi : p.npad, :]
                for s0, nsg, chunk_ids, _tr in p.groups:
                    # one PSUM bank per supertile: windows of a supertile sit
                    # on disjoint partitions, so their has_written groups can
                    # interleave freely; supertiles never share a bank
                    aggs = [
                        psagg.tile([128, HID], f32, tag="agg", name="agg")
                        for _ in range(nsg)
                    ]
                    for ci in chunk_ids:
                        t0, ntl, cls = p.chunks[ci]
                        gt = gpool.tile([128, CHUNK_MAX, HID], f16, tag="gath")
                        view = hi_view if cls == 1 else lo_view
                        nc.gpsimd.dma_gather(
                            gt[:, :ntl, :],
                            view,
                            idx_sb[:, t0 * 8 : (t0 + ntl) * 8],
                            ntl * 128,
                            ntl * 128,
                            HID,
                            elem_step=HID,
                            single_packet=SINGLE_PACKET,
                            queue_num=ci % N_QUEUES,
                        )
                        st = spool.tile([128, SUP, CHUNK_MAX], f16, tag="sbld")
                        nc.vector.tensor_tensor(
                            st[:, :, :ntl],
                            iota[:, :, :ntl],
                            bcmid(dstslot[:, t0 : t0 + ntl], SUP),
                            Alu.is_equal,
                        )
                        nc.vector.tensor_tensor(
                            st[:, :, :ntl],
                            st[:, :, :ntl],
                            bcmid(ews[:, t0 : t0 + ntl], SUP),
                            Alu.mult,
                        )
                        for j in range(ntl):
                            t = t0 + j
                            s_abs, _cls = p.tiles[t]
                            sj = s_abs - s0
                            nc.tensor.matmul(
                                aggs[sj][0:fdim, :],
                                gt[:, j, 0:fdim],
                                st[:, :, j],
                                start=bool(p.first_of_sup[t]),
                                stop=bool(p.last_of_sup[t]),
                                skip_group_check=True,
                            )
                    for sj in range(nsg):
                        s = s0 + sj
                        psum_agg = aggs[sj][0:fdim, :]
                        rep = dinv_rep[:, s * 128 : (s + 1) * 128]
                        hinT = hin[:, s * 128 : (s + 1) * 128]
                        stg = psstg.tile([128, 320], f32, tag="stg")
                        hTps = stg[:, 0:128]
                        trps = stg[:, 128:256]
                        uT = stpool.tile([128, 128], f32, tag="uTs")
                        # uT = (aggT + h*dinv|T) * dinv(col)
                        nc.vector.tensor_tensor(
                            uT[0:fdim, :], psum_agg, hinT, Alu.add
                        )
                        nc.vector.tensor_tensor(
                            uT[0:fdim, :], uT[0:fdim, :], rep[0:fdim, :], Alu.mult
                        )
                        nc.tensor.matmul(
                            hTps,
                            w_sb[0:fdim, :],
                            uT[0:fdim, :],
                            start=True,
                            stop=True,
                        )
                        hT = stpool.tile([128, 128], f32, tag="hTs")
                        if relu:
                            nc.scalar.activation(
                                hT[:, :],
                                hTps,
                                Act.Relu,
                                bias=b_sb[:, 0:1],
                            )
                        else:
                            nc.vector.tensor_scalar(
                                hT[:, :], hTps, b_sb[:, 0:1], None, Alu.add
                            )
                        if li < 2:
                            hfT = hfout[:, s * 128 : (s + 1) * 128]
                            nc.vector.tensor_tensor(hfT, hT[:, :], rep, Alu.mult)
                            nc.tensor.transpose(trps, hT[:, :], ident[:, :])
                            hf = stpool.tile([128, 128], f16, tag="hf")
                            nc.scalar.activation(
                                hf[:, :],
                                trps,
                                Act.Copy,
                                scale=dinv_own[:, s : s + 1],
                            )
                            nc.sync.dma_start(
                                agin_d[:, :].rearrange("(t q) f -> q t f", q=128)[
                                    :, s, :
                                ],
                                hf[:, :],
                            )
                            if tab_next is not None and (s + 1) in ag_ends:
                                k = ag_ends.index(s + 1)
                                r0, r1 = p.ag_splits[k]
                                base = p.ag_base[k]
                                blen = p.n_cores * (r1 - r0) * SUP
                                nc.gpsimd.collective_compute(
                                    "AllGather",
                                    Alu.bypass,
                                    replica_groups=rg,
                                    ins=[agin_d[r0 * SUP : r1 * SUP, :]],
                                    outs=[tab_next[base : base + blen, :]],
                                )
                        else:
                            nc.tensor.transpose(trps, hT[:, :], ident[:, :])
                            h3 = stpool.tile([128, 128], f16, tag="hf")
                            nc.vector.tensor_copy(h3[:, :], trps)
                            nc.tensor.matmul(
                                pacc[:, 0:G],
                                h3[:, :],
                                pmat[:, s * G : (s + 1) * G],
                                start=(s == 0),
                                stop=(s == p.nsup - 1),
                                skip_group_check=True,
                            )


            # ---- pooling finalize + classifier ----
            pooledT = stpool.tile([128, G], f32, tag="pool")
            nc.vector.tensor_copy(pooledT[:, :], pacc[:, 0:G])
            nc.sync.dma_start(arin_d[:, :], pooledT[:, :])
            nc.gpsimd.collective_compute(
                "AllReduce",
                Alu.add,
                replica_groups=rg,
                ins=[arin_d[:, :]],
                outs=[arout_d[:, :]],
            )
            pooled2 = stpool.tile([128, G], f32, tag="pool")
            nc.sync.dma_start(pooled2[:, :], arout_d[:, :])
            lgps = psstg.tile([128, 320], f32, tag="stg")
            nc.tensor.matmul(
                lgps[0:G, 0:N_CLASS], pooled2[:, :], wl[:, :], start=True, stop=True
            )
            outt = stpool.tile([G, N_CLASS], f32, tag="out")
            nc.vector.scalar_tensor_tensor(
                outt[:, :],
                lgps[0:G, 0:N_CLASS],
                invc[:, 0:1],
                blrep[:, :],
                Alu.mult,
                Alu.add,
            )
            nc.sync.dma_start(out_d[:, :], outt[:, :])

    nc.compile()
    return nc


def make_in_maps(p, wa):
    maps = []
    for c in range(p.n_cores):
        pc = p.per_core[c]
        maps.append(
            dict(
                x_own=p.x_own[c],
                ewp=pc["ewp"].reshape(128, p.nsup, p.L),
                idx=pc["idx128"],
                dstslot=pc["dstslot"],
                ews=pc["ews"],
                iota=p.iota.reshape(128, SUP, CHUNK_MAX),
                pmat=p.pmat[c],
                w1=wa["w1"],
                w2=wa["w2"],
                w3=wa["w3"],
                wl=wa["wl"],
                b1=wa["b1"],
                b2=wa["b2"],
                b3=wa["b3"],
                invc=wa["invc"],
                blrep=wa["blrep"],
                ident=p.identity,
            )
        )
    return maps


_CACHE = {}


def kernel(x, edge_index, edge_attr, batch, W1, b1, W2, b2, W3, b3, Wl, bl):
    x = np.asarray(x)
    p = build_plan(x, np.asarray(edge_index), np.asarray(edge_attr), np.asarray(batch))
    wa = build_weight_arrays(p, W1, b1, W2, b2, W3, b3, Wl, bl)
    key = (p.n, p.tot)
    if key not in _CACHE:
        _CACHE[key] = build_program(p)
    nc = _CACHE[key]
    from concourse.bass_utils import run_bass_kernel_spmd

    res = run_bass_kernel_spmd(nc, make_in_maps(p, wa), core_ids=list(range(p.n_cores)))
    return np.asarray(res.results[0]["out"], dtype=np.float32)

